# revision 9
# baseline (speedup 1.0000x reference)
"""BinsChamferLoss Trainium2 kernel (v2).

Problem: bins [4,257], target_depth_maps [4,240,320] ->
scalar chamfer loss between per-image bin centers (256 1-D points) and
the valid depth pixels (76800 1-D points per image).

Sharding: the 76800-pixel dim is split across 8 cores (9600 pixels each),
all 4 images and all 256 bins on every core. Host combine is a tiny
min/sum over per-core partials.

Per-core device pipeline (v2):
  cham_y ([part=points, free=point-stream], partition p owns batch p//32,
          300 points per partition):
    a chained min over 128 bin-PAIRS: one custom DVE op per pair with
    body = min(min((t-bc_a)^2, (t-bc_b)^2), dy_prev), streaming all 300
    points per instruction. 128 instrs x (58+300) cyc ~= 48us, vs 300
    small per-column ops (~76us+) in v1.
  cham_x ([part=bins, free=points]):
    t broadcast to [128, 9600] in BF16 (halves DMA), ACT computes
    Square(t + (-bc_p)) -> bf16 dsq, stock DVE tensor_reduce(min) per
    (batch, chunk) reduces at 2-4 elem/cycle (bf16 perf modes) instead
    of a custom 1x op. ACT ~66us || DVE ~70us || DMA ~30us.
Invalid points (t < 0.001) are pushed 1e9 away so they never win a min
and their dy contribution is masked out of the sum.
"""

import os
import sys

import numpy as np

sys.path.insert(0, "/opt/trn_rl_repo")

N_CORES = 8
N, P = 4, 256  # batches, bins
L = 240 * 320  # 76800 points per batch
L_LOC = L // N_CORES  # 9600 per core
COLS = (N * L_LOC) // 128  # 300 point-columns per partition
PARTS_PER_BATCH = 128 // N  # 32
_CACHE = {}

# bisect switches (dev only; default = full v2)
CHAMX_MODE = os.environ.get("CHAMX_MODE", "v2")
CHAMY_MODE = os.environ.get("CHAMY_MODE", "v2")
# tensor_tensor_reduce (TAIL_MODE=v2) crashes the exec unit on this runtime
# (NRT_EXEC_UNIT_UNRECOVERABLE) — keep the mul+reduce tail.
TAIL_MODE = os.environ.get("TAIL_MODE", "v1")


def _register(name, spec):
    """Register (idempotently) a custom DVE op from a Spec."""
    from concourse.dve_ops import (CUSTOM_DVE_SPECS, OPS,
                                   _SUB_OPCODE_FOR_NAME, DveOp, has_src1)
    from concourse.dve_spec import lower
    from concourse.dve_uop import DveOpSpec

    if name in _SUB_OPCODE_FOR_NAME:
        return next(o for o in OPS if o.name == name)
    row = 1 + len(OPS)
    shas = {}
    for ver in ("v3", "v4"):
        s = DveOpSpec(name=name, opcode=row, uops=lower(spec, ver=ver),
                      rd1_en=has_src1(spec))
        shas[ver] = s.sha(ver)
    _SUB_OPCODE_FOR_NAME[name] = row
    op = DveOp(name, spec, subdim=False, uops_sha=shas)
    OPS.append(op)
    CUSTOM_DVE_SPECS[name] = spec
    return op


def _pair_ref(in0, in1, c0, c1, c2):
    c0 = np.asarray(c0, np.float32).reshape(-1, 1)
    c1 = np.asarray(c1, np.float32).reshape(-1, 1)
    x = in0.astype(np.float32)
    return np.minimum((x - c0) ** 2, (x - c1) ** 2).astype(np.float32)


def _chain_ref(in0, in1, c0, c1, c2):
    c0 = np.asarray(c0, np.float32).reshape(-1, 1)
    c1 = np.asarray(c1, np.float32).reshape(-1, 1)
    x = in0.astype(np.float32)
    pair = np.minimum((x - c0) ** 2, (x - c1) ** 2)
    return np.minimum(pair, in1.astype(np.float32)).astype(np.float32)


def _chamy_ops():
    """(pair_op, chain_op): pair = min of sq-dists to two bins;
    chain = same then min with the running dy stream."""
    from concourse.dve_spec import C0, C1, Spec, Src0, Src1, minn, sq

    pair = _register("CHAMY_PAIR_ANT",
                     Spec(body=minn(sq(Src0 - C0), sq(Src0 - C1)),
                          reference=_pair_ref))
    chain = _register("CHAMY_CHAIN_ANT",
                      Spec(body=minn(minn(sq(Src0 - C0), sq(Src0 - C1)),
                                     Src1),
                           reference=_chain_ref))
    return pair, chain


def _chamy_v1_ref(in0, in1, c0, c1, c2):
    c0 = np.asarray(c0, np.float32).reshape(-1, 1)
    P_ = in0.shape[0]
    a = (in0.astype(np.float32).reshape(P_, -1) - c0) ** 2
    b = (in1.astype(np.float32).reshape(P_, -1) - c0) ** 2
    body = np.minimum(a, b).astype(np.float32)
    c1 = np.asarray(c1, np.float32).reshape(-1, 1)
    acc = np.minimum(body.min(axis=-1, keepdims=True), c1)
    return body.reshape(in0.shape), acc


def _min2_ref(in0, in1, c0, c1, c2):
    P_ = in0.shape[0]
    body = np.minimum(in0.astype(np.float32),
                      in1.astype(np.float32)).astype(np.float32)
    b2 = body.reshape(P_, -1)
    c1 = np.asarray(c1, np.float32).reshape(-1, 1)
    acc = np.minimum(b2.min(axis=-1, keepdims=True), c1)
    return body, acc


def _v1_ops():
    from concourse.dve_spec import C0, C1, Spec, Src0, Src1, minn, sq

    chamy = _register("CHAMY2_SQDIFF_MINRED_ANT",
                      Spec(body=minn(sq(Src0 - C0), sq(Src1 - C0)),
                           accum=minn, accum_init=C1,
                           reference=_chamy_v1_ref))
    min2 = _register("MIN2_MINRED_ANT",
                     Spec(body=minn(Src0, Src1), accum=minn, accum_init=C1,
                          reference=_min2_ref))
    return chamy, min2


def _body(nc, tc, tile, mybir, tpd, bct, bcn, outx, outy):
    f32 = mybir.dt.float32
    bf16 = mybir.dt.bfloat16
    fp16 = mybir.dt.float16
    Alu = mybir.AluOpType
    Act = mybir.ActivationFunctionType
    X = mybir.AxisListType.X

    pair_op, chain_op = _chamy_ops()
    chamy1_op, min2_op = _v1_ops()

    with tc.tile_pool(name="consts", bufs=1) as consts, \
         tc.tile_pool(name="work", bufs=4) as work, \
         tc.tile_pool(name="bcast", bufs=2) as bcast, \
         tc.tile_pool(name="dsqp", bufs=3) as dsqp:
        bct_sb = consts.tile([128, P], f32, tag="bct")
        nc.sync.dma_start(bct_sb[:], bct)
        bcn_sb = consts.tile([128, 2 * N], f32, tag="bcn")
        nc.sync.dma_start(bcn_sb[:], bcn)

        tp_sb = consts.tile([128, COLS], f32, tag="tp")
        tpd_pc = tpd.rearrange("(p c) -> p c", p=128)
        nc.sync.dma_start(tp_sb[:], tpd_pc)

        # valid = (t >= 0.001); t_adj = t + (1-valid)*1e9
        valid = consts.tile([128, COLS], f32, tag="valid")
        nc.vector.tensor_scalar(valid[:], tp_sb[:], 0.001, None,
                                op0=Alu.is_ge)
        tmp = consts.tile([128, COLS], f32, tag="tmp")
        nc.vector.tensor_scalar(tmp[:], valid[:], -1e9, 1e9,
                                op0=Alu.mult, op1=Alu.add)
        t_adj = consts.tile([128, COLS], f32, tag="tadj")
        nc.vector.tensor_add(t_adj[:], tmp[:], tp_sb[:])
        # fp16 copy of t_adj for the cham_x broadcast path (fp16: 10-bit
        # mantissa; bf16's 8-bit snaps points to a ~2e-3 grid and inflates
        # the per-bin min distance)
        tbf = consts.tile([128, COLS], fp16, tag="tbf")
        nc.vector.tensor_copy(tbf[:], t_adj[:])

        chx = consts.tile([128, 2 * N], f32, tag="chx")

        if CHAMX_MODE == "v2":
            # DRAM bounce ([128,300] -> flat [4,9600]) then per-batch
            # partition-broadcast, all in bf16
            tscratch = nc.dram_tensor("tscratch", [N * L_LOC], fp16,
                                      kind="Internal").ap()
            nc.sync.dma_start(tscratch.rearrange("(p c) -> p c", p=128),
                              tbf[:])
            # ---- cham_x: ACT squares, stock DVE min-reduce ----
            for n in range(N):
                tbc = bcast.tile([128, L_LOC], fp16, tag="tbc")
                nc.sync.dma_start(
                    tbc[:], tscratch[n * L_LOC:(n + 1) * L_LOC]
                    .partition_broadcast(128))
                for c in range(2):
                    dsq = dsqp.tile([128, L_LOC], bf16, tag="dsq")
                    nc.scalar.activation(dsq[:], tbc[:], Act.Square,
                                         bias=bcn_sb[:, n * 2 + c:
                                                     n * 2 + c + 1],
                                         scale=1.0)
                    nc.vector.tensor_reduce(chx[:, n * 2 + c:n * 2 + c + 1],
                                            dsq[:], axis=X, op=Alu.min)
        else:
            # v1 path: f32 broadcast, ACT f32->bf16 square, custom min2
            tscratch = nc.dram_tensor("tscratch", [N * L_LOC], f32,
                                      kind="Internal").ap()
            nc.sync.dma_start(tscratch.rearrange("(p c) -> p c", p=128),
                              t_adj[:])
            H = L_LOC // 2
            for n in range(N):
                tbc = bcast.tile([128, L_LOC], f32, tag="tbc")
                nc.sync.dma_start(
                    tbc[:], tscratch[n * L_LOC:(n + 1) * L_LOC]
                    .partition_broadcast(128))
                for c in range(2):
                    dsq = dsqp.tile([128, L_LOC], bf16, tag="dsq")
                    nc.scalar.activation(dsq[:], tbc[:], Act.Square,
                                         bias=bcn_sb[:, n * 2 + c:
                                                     n * 2 + c + 1],
                                         scale=1.0)
                    tr1 = dsqp.tile([128, H], bf16, tag="tr1")
                    nc.vector._custom_dve(
                        min2_op, out=tr1[:], in0=dsq[:, 0:H],
                        in1=dsq[:, H:L_LOC], s1=3.0e38,
                        accum_out=chx[:, n * 2 + c:n * 2 + c + 1])

        if CHAMY_MODE == "v2":
            # ---- cham_y: chained min over 128 bin-pairs ----
            dyA = consts.tile([128, COLS], f32, tag="dyA")
            dyB = consts.tile([128, COLS], f32, tag="dyB")
            nc.vector._custom_dve(pair_op, out=dyA[:], in0=t_adj[:],
                                  s0=bct_sb[:, 0:1], s1=bct_sb[:, 1:2])
            bufs = [dyA, dyB]
            for s in range(1, P // 2):
                src = bufs[(s + 1) % 2]
                dst = bufs[s % 2]
                nc.vector._custom_dve(chain_op, out=dst[:], in0=t_adj[:],
                                      in1=src[:],
                                      s0=bct_sb[:, 2 * s:2 * s + 1],
                                      s1=bct_sb[:, 2 * s + 1:2 * s + 2])
            dy_last = bufs[(P // 2 - 1) % 2]
            dy_ap = dy_last[:]
        else:
            # v1 path: one dual-stream op per point column
            dy = consts.tile([128, COLS], f32, tag="dy")
            for j in range(COLS):
                scr = work.tile([128, P // 2], f32, tag="scr")
                nc.vector._custom_dve(chamy1_op, out=scr[:],
                                      in0=bct_sb[:, 0:P // 2],
                                      in1=bct_sb[:, P // 2:P],
                                      s0=tp_sb[:, j:j + 1], s1=3.0e38,
                                      accum_out=dy[:, j:j + 1])
            dy_ap = dy[:]

        # dy * valid, summed; plus valid count
        osum = consts.tile([128, 2], f32, tag="osum")
        if TAIL_MODE == "v2":
            dym = consts.tile([128, COLS], f32, tag="dym")
            nc.vector.tensor_tensor_reduce(out=dym[:], in0=dy_ap,
                                           in1=valid[:], scale=1.0,
                                           scalar=0.0,
                                           op0=Alu.mult, op1=Alu.add,
                                           accum_out=osum[:, 0:1])
        else:
            dym = consts.tile([128, COLS], f32, tag="dym")
            nc.vector.tensor_mul(dym[:], dy_ap, valid[:])
            nc.vector.tensor_reduce(osum[:, 0:1], dym[:], axis=X, op=Alu.add)
        nc.vector.tensor_reduce(osum[:, 1:2], valid[:], axis=X, op=Alu.add)

        # outputs on the SWDGE path so they never block the sync queue
        nc.gpsimd.dma_start(outx, chx[:])
        nc.gpsimd.dma_start(outy, osum[:])


def _build_program():
    import concourse.bacc as bacc
    import concourse.tile as tile
    from concourse import mybir

    f32 = mybir.dt.float32

    nc = bacc.Bacc("TRN2", target_bir_lowering=False, debug=False,
                   num_devices=N_CORES)
    tpd = nc.dram_tensor("tpd", [N * L_LOC], f32, kind="ExternalInput").ap()
    bct = nc.dram_tensor("bct", [128, P], f32, kind="ExternalInput").ap()
    bcn = nc.dram_tensor("bcn", [128, 2 * N], f32, kind="ExternalInput").ap()
    outx = nc.dram_tensor("outx", [128, 2 * N], f32,
                          kind="ExternalOutput").ap()
    outy = nc.dram_tensor("outy", [128, 2], f32, kind="ExternalOutput").ap()

    with tile.TileContext(nc) as tc:
        _body(nc, tc, tile, mybir, tpd, bct, bcn, outx, outy)
    nc.compile()
    return nc


def _get_program():
    if "nc" not in _CACHE:
        _CACHE["nc"] = _build_program()
    return _CACHE["nc"]


def make_inputs(bins, target_depth_maps):
    bins = np.asarray(bins, dtype=np.float32)
    tdm = np.asarray(target_depth_maps, dtype=np.float32)
    bc = 0.5 * (bins[:, 1:] + bins[:, :-1])  # [4, 256]
    bct = np.ascontiguousarray(bc[np.arange(128) // PARTS_PER_BATCH])
    # bcn[p, n*2+c] = -bc[n, c*128+p]
    bcn = np.empty((128, 2 * N), dtype=np.float32)
    for n in range(N):
        for c in range(2):
            bcn[:, n * 2 + c] = -bc[n, c * 128:(c + 1) * 128]
    tp = tdm.reshape(N, L)
    in_maps = []
    for c in range(N_CORES):
        shard = np.ascontiguousarray(
            tp[:, c * L_LOC:(c + 1) * L_LOC]).reshape(-1)
        in_maps.append({"tpd": shard, "bct": bct, "bcn": bcn})
    return in_maps


def combine(outs):
    accx = np.stack([o["outx"] for o in outs])  # [8, 128, 2N]
    osum = np.stack([o["outy"] for o in outs])  # [8, 128, 2]
    total = np.float64(0.0)
    for n in range(N):
        # cham_x: min over cores of per-bin d^2 mins, both chunks
        mins = accx[:, :, n * 2:n * 2 + 2].min(axis=0)  # [128, 2]
        cham_x = mins.mean()
        sl = slice(n * PARTS_PER_BATCH, (n + 1) * PARTS_PER_BATCH)
        dsum = osum[:, sl, 0].sum()
        cnt = osum[:, sl, 1].sum()
        cham_y = dsum / cnt
        total += cham_x + cham_y
    return np.array(total / N, dtype=np.float32)


def kernel(bins, target_depth_maps):
    from concourse.bass_utils import run_bass_kernel_spmd

    in_maps = make_inputs(bins, target_depth_maps)
    nc = _get_program()
    res = run_bass_kernel_spmd(nc, in_maps, core_ids=list(range(N_CORES)))
    return combine(res.results)
